# revision 1
# baseline (speedup 1.0000x reference)
"""Multi-head self-attention (B=4, S=2048, E=1024, H=16, causal) on 8 NeuronCores.

Sharding (Megatron-style, per hint): data-parallel over B (4) x tensor-parallel
over heads (2 groups of 8 heads). Core c handles batch c//2 with head-group
c%2: Wq/Wk/Wv sharded column-wise, Wo row-wise. Each core emits a partial
out-projection [S, E]; the host sums each pair of partials (the "all-reduce")
and adds bo.

Per-core kernel (all matmuls bf16, fp32 accumulation):
  - host supplies x[b].T so Q^T,K^T ([d,s]) and V ([s,d]) come straight off
    the projections with no on-chip transposes
  - scores computed transposed (S^T = K Q^T, [keys, queries]) with causal
    block-skipping; exp on ScalarE with fused 1/sqrt(D) scale reads PSUM
    directly and writes bf16
  - softmax denominator comes free from a ones-column appended to V in the
    attn @ V matmul; normalization is deferred to the [d, s] attention output
    (rank-1 PE broadcast of 1/sum + one vector multiply), where the V-bias
    also reduces to a per-partition add
"""

import numpy as np
import ml_dtypes

B, S, E, H, D = 4, 2048, 1024, 16, 64
HPC = 8          # heads per core
DC = HPC * D     # 512 sharded feature cols per core
EC = E // 128    # 8 e-chunks
TT = S // 128    # 16 token tiles
QCH = S // 512   # 4 query chunks
NB = S // 128    # 16 key blocks

BF16 = ml_dtypes.bfloat16

_CACHE = {}


def _build():
    import concourse.tile as tile
    from concourse import bacc, mybir

    F32 = mybir.dt.float32
    BF = mybir.dt.bfloat16
    AF = mybir.ActivationFunctionType
    ALU = mybir.AluOpType

    nc = bacc.Bacc("TRN2", target_bir_lowering=False, debug=False, num_devices=8)

    xT_d = nc.dram_tensor("xT", [EC, 128, S], BF, kind="ExternalInput")
    wq_d = nc.dram_tensor("wq", [EC, 128, DC], BF, kind="ExternalInput")
    wk_d = nc.dram_tensor("wk", [EC, 128, DC], BF, kind="ExternalInput")
    wv_d = nc.dram_tensor("wv", [EC, 128, DC], BF, kind="ExternalInput")
    wo_d = nc.dram_tensor("wo", [DC // 128, 128, E], BF, kind="ExternalInput")
    bq_d = nc.dram_tensor("bq", [128, 4], F32, kind="ExternalInput")
    bk_d = nc.dram_tensor("bk", [128, 4], F32, kind="ExternalInput")
    bv_d = nc.dram_tensor("bv", [128, 4], F32, kind="ExternalInput")
    mask_d = nc.dram_tensor("mask", [128, 128], BF, kind="ExternalInput")
    out_d = nc.dram_tensor("out", [TT, 128, E], F32, kind="ExternalOutput")

    with tile.TileContext(nc) as tc:
        with tc.tile_pool(name="const", bufs=1) as cp, \
             tc.tile_pool(name="expp", bufs=1) as expp, \
             tc.tile_pool(name="work", bufs=2) as wp, \
             tc.tile_pool(name="ps_s", bufs=2, space="PSUM") as ps_s, \
             tc.tile_pool(name="ps_av", bufs=2, space="PSUM") as ps_av, \
             tc.tile_pool(name="ps_w", bufs=2, space="PSUM") as ps_w:

            # ---- persistent SBUF tensors + input DMAs ----
            xT = [cp.tile([128, S], BF, tag=f"xT{k}", name=f"xT{k}") for k in range(EC)]
            wq = [cp.tile([128, DC], BF, tag=f"wq{k}", name=f"wq{k}") for k in range(EC)]
            wk = [cp.tile([128, DC], BF, tag=f"wk{k}", name=f"wk{k}") for k in range(EC)]
            wv = [cp.tile([128, DC], BF, tag=f"wv{k}", name=f"wv{k}") for k in range(EC)]
            wo = [cp.tile([128, E], BF, tag=f"wo{k}", name=f"wo{k}") for k in range(DC // 128)]
            for k in range(EC):
                nc.sync.dma_start(xT[k][:], xT_d.ap()[k])
                nc.gpsimd.dma_start(wq[k][:], wq_d.ap()[k])
                nc.gpsimd.dma_start(wk[k][:], wk_d.ap()[k])
                nc.gpsimd.dma_start(wv[k][:], wv_d.ap()[k])
            for k in range(DC // 128):
                nc.sync.dma_start(wo[k][:], wo_d.ap()[k])
            bq = cp.tile([128, 4], F32, tag="bq", name="bq")
            bk = cp.tile([128, 4], F32, tag="bk", name="bk")
            bv = cp.tile([128, 4], F32, tag="bv", name="bv")
            mask = cp.tile([128, 128], BF, tag="mask", name="mask")
            nc.sync.dma_start(bq[:], bq_d.ap())
            nc.sync.dma_start(bk[:], bk_d.ap())
            nc.sync.dma_start(bv[:], bv_d.ap())
            nc.sync.dma_start(mask[:], mask_d.ap())
            ones = cp.tile([65, 64], BF, tag="ones", name="ones")
            nc.any.memset(ones[:], 1.0)

            QT = [cp.tile([128, S], BF, tag=f"QT{t}", name=f"QT{t}") for t in range(4)]
            KT = [cp.tile([128, S], BF, tag=f"KT{t}", name=f"KT{t}") for t in range(4)]
            V = [cp.tile([128, HPC, 66], BF, tag=f"V{s}", name=f"V{s}") for s in range(TT)]
            AOT = [cp.tile([128, S], BF, tag=f"AOT{t}", name=f"AOT{t}") for t in range(4)]

            # Projection work is emitted as "filler" interleaved into the
            # attention stream: attention is ScalarE(exp)-paced with PE half
            # idle, while projections are pure dense PE - mixing them keeps
            # every engine busy. Fillers are forced ahead of their consumers.
            filler = []

            def proj_group(w_sb, b_sb, dst, t, qc):
                def emit():
                    ps = ps_w.tile([128, 512], F32, tag="psw", name="psw")
                    for k in range(EC):
                        nc.tensor.matmul(
                            ps[:],
                            w_sb[k][:, t * 128:(t + 1) * 128],
                            xT[k][:, qc * 512:(qc + 1) * 512],
                            start=(k == 0), stop=(k == EC - 1))
                    nc.scalar.activation(
                        dst[t][:, qc * 512:(qc + 1) * 512], ps[:],
                        AF.Identity, bias=b_sb[:, t:t + 1], scale=1.0)
                return emit

            def v_group(s):
                def emit():
                    ps = ps_w.tile([128, 512], F32, tag="psw", name="psw")
                    for k in range(EC):
                        nc.tensor.matmul(
                            ps[:],
                            xT[k][:, s * 128:(s + 1) * 128],
                            wv[k][:],
                            start=(k == 0), stop=(k == EC - 1))
                    nc.vector.tensor_copy(
                        out=V[s][:, :, 0:64],
                        in_=ps[:].rearrange("p (h d) -> p h d", d=64))
                    nc.any.memset(V[s][:, :, 64:65], 1.0)
                return emit

            def d_group(s):
                def emit():
                    osb = wp.tile([128, E], F32, tag="osb", name="osb")
                    for n in range(2):
                        ps = ps_w.tile([128, 512], F32, tag="psw", name="psw")
                        for k in range(DC // 128):
                            nc.tensor.matmul(
                                ps[:],
                                AOT[k][:, s * 128:(s + 1) * 128],
                                wo[k][:, n * 512:(n + 1) * 512],
                                start=(k == 0), stop=(k == DC // 128 - 1))
                        nc.vector.tensor_copy(out=osb[:, n * 512:(n + 1) * 512],
                                              in_=ps[:])
                    nc.sync.dma_start(out_d.ap()[s], osb[:])
                return emit

            # up-front: pair-0 projections + first V tiles (attention prologue)
            for t in range(4):
                for qc in range(QCH):
                    if t == 0:
                        proj_group(wq, bq, QT, t, qc)()
                        proj_group(wk, bk, KT, t, qc)()
                    else:
                        filler.append(("qkt", t, proj_group(wq, bq, QT, t, qc)))
                        filler.append(("qkt", t, proj_group(wk, bk, KT, t, qc)))
            for s in range(TT):
                if s < 4:
                    v_group(s)()
                else:
                    filler.append(("v", s, v_group(s)))

            def emit_filler_until(pred_drop):
                """Emit (and drop) all fillers matching pred_drop."""
                keep = []
                for item in filler:
                    if pred_drop(item):
                        item[2]()
                    else:
                        keep.append(item)
                filler[:] = keep

            def emit_some_filler(n):
                for _ in range(min(n, len(filler))):
                    filler.pop(0)[2]()

            # ---- attention, head-pair interleaved, qc-outer ----
            # qc=3 (the longest phase, 32 rounds) runs first so it absorbs
            # the projection fillers; V tiles are forced in per-round just
            # before the key block that consumes them.
            for qc in (3, 0, 1, 2):
                nkb = 4 * qc + 4
                for hp in range(4):
                    emit_filler_until(lambda it: it[0] == "qkt" and it[1] <= hp)
                    hA, hB = 2 * hp, 2 * hp + 1
                    pav = {}
                    expT = {}
                    pav[hA] = ps_av.tile([128, 512], F32, tag="pav", name="pav")
                    pav[hB] = ps_av.tile([128, 512], F32, tag="pav", name="pav")
                    expT[hA] = expp.tile([128, NB, 512], BF, tag="expTA",
                                         name="expTA")
                    expT[hB] = expp.tile([128, NB, 512], BF, tag="expTB",
                                         name="expTB")
                    def emit_av(kbs_offs):
                        for h in (hA, hB):
                            for kb, off in kbs_offs:
                                nc.tensor.matmul(
                                    pav[h][0:65, off:512],
                                    V[kb][:, h, 0:65],
                                    expT[h][:, kb, off:512],
                                    start=(kb == 0), stop=(kb == nkb - 1))

                    for s0 in range(0, nkb, 2):
                        kbs = list(range(s0, min(s0 + 2, nkb)))
                        emit_filler_until(
                            lambda it: it[0] == "v" and it[1] <= kbs[-1])
                        pss = {h: ps_s.tile([128, 2, 512], F32, tag="pss",
                                            name="pss")
                               for h in (hA, hB)}
                        offs = {}
                        for i, kb in enumerate(kbs):
                            dj = kb - 4 * qc
                            off = 128 * dj if dj > 0 else 0
                            offs[kb] = off
                            for h, r in ((hA, 0), (hB, 64)):
                                nc.tensor.matmul(
                                    pss[h][:, i, off:512],
                                    KT[hp][r:r + 64, kb * 128:(kb + 1) * 128],
                                    QT[hp][r:r + 64,
                                           qc * 512 + off:(qc + 1) * 512],
                                    start=True, stop=True)
                        for h in (hA, hB):
                            if kbs[-1] < 4 * qc:
                                nc.scalar.activation(
                                    expT[h][:, s0:s0 + len(kbs), :],
                                    pss[h][:, 0:len(kbs), :],
                                    AF.Exp, scale=0.125)
                            else:
                                for i, kb in enumerate(kbs):
                                    dj = kb - 4 * qc
                                    off = offs[kb]
                                    nc.scalar.activation(
                                        expT[h][:, kb, off:512],
                                        pss[h][:, i, off:512],
                                        AF.Exp, scale=0.125)
                                    if dj >= 0:
                                        nc.vector.tensor_tensor(
                                            expT[h][:, kb, off:off + 128],
                                            expT[h][:, kb, off:off + 128],
                                            mask[:], ALU.mult)
                        emit_av([(kb, offs[kb]) for kb in kbs])
                        emit_some_filler(1)
                    # normalization + V-bias per head
                    for h, r in ((hA, 0), (hB, 64)):
                        rcp = wp.tile([65, 512], BF, tag="rcp", name="rcp")
                        with nc.allow_low_precision("softmax denom bcast"):
                            nc.vector.reciprocal(out=rcp[64:65, :],
                                                 in_=pav[h][64:65, :])
                        psb = ps_w.tile([128, 512], F32, tag="psw", name="psw")
                        nc.tensor.matmul(psb[0:64, :], ones[64:65, :],
                                         rcp[64:65, :], start=True, stop=True)
                        binv = wp.tile([64, 512], F32, tag="binv", name="binv")
                        nc.vector.tensor_copy(out=binv[:], in_=psb[0:64, :])
                        dst = AOT[hp][r:r + 64, qc * 512:(qc + 1) * 512]
                        nc.vector.tensor_tensor(dst, pav[h][0:64, :], binv[:],
                                                ALU.mult)
                        nc.vector.tensor_scalar(dst, dst, bv[r:r + 64,
                                                             hp:hp + 1],
                                                None, ALU.add)
                # out-projection for this qc becomes filler for qc+1
                for s in range(qc * 4, qc * 4 + 4):
                    filler.append(("d", s, d_group(s)))
            emit_filler_until(lambda it: True)

    nc.compile()
    return nc


def _get_nc():
    if "nc" not in _CACHE:
        _CACHE["nc"] = _build()
    return _CACHE["nc"]


def _shard_inputs(x, Wq, bq, Wk, bk, Wv, bv, Wo):
    """Build the 8 per-core input maps (host-side shard/cast/transpose)."""
    x = np.asarray(x, np.float32)
    mask = np.triu(np.ones((128, 128), np.float32)).astype(BF16)  # [k, q] q>=k
    in_maps = []
    for c in range(8):
        b, hg = divmod(c, 2)
        dc = slice(hg * DC, (hg + 1) * DC)
        xT = np.ascontiguousarray(x[b].T).astype(BF16).reshape(EC, 128, S)
        wq_c = np.ascontiguousarray(Wq[:, dc]).astype(BF16).reshape(EC, 128, DC)
        wk_c = np.ascontiguousarray(Wk[:, dc]).astype(BF16).reshape(EC, 128, DC)
        wv_c = np.ascontiguousarray(Wv[:, dc]).astype(BF16).reshape(EC, 128, DC)
        wo_c = np.ascontiguousarray(Wo[dc, :]).astype(BF16).reshape(DC // 128, 128, E)
        bq_c = np.ascontiguousarray(np.asarray(bq[dc], np.float32).reshape(4, 128).T)
        bk_c = np.ascontiguousarray(np.asarray(bk[dc], np.float32).reshape(4, 128).T)
        bv_c = np.ascontiguousarray(np.asarray(bv[dc], np.float32).reshape(4, 128).T)
        in_maps.append({
            "xT": xT, "wq": wq_c, "wk": wk_c, "wv": wv_c, "wo": wo_c,
            "bq": bq_c, "bk": bk_c, "bv": bv_c, "mask": mask,
        })
    return in_maps


def kernel(x, Wq, bq, Wk, bk, Wv, bv, Wo, bo):
    from concourse.bass_utils import run_bass_kernel_spmd

    nc = _get_nc()
    in_maps = _shard_inputs(x, Wq, bq, Wk, bk, Wv, bv, Wo)
    res = run_bass_kernel_spmd(nc, in_maps, core_ids=list(range(8)))
    bo = np.asarray(bo, np.float32)
    out = np.empty((B, S, E), np.float32)
    for b in range(B):
        p0 = res.results[2 * b]["out"].reshape(S, E)
        p1 = res.results[2 * b + 1]["out"].reshape(S, E)
        out[b] = p0 + p1 + bo
    return out



# revision 4
# speedup vs baseline: 1.1860x; 1.1860x over previous
"""Multi-head self-attention (B=4, S=2048, E=1024, H=16, causal) on 8 NeuronCores.

Sharding (Megatron-style, per hint): data-parallel over B (4) x tensor-parallel
over heads (2 groups of 8 heads). Core c handles batch c//2 with head-group
c%2: Wq/Wk/Wv sharded column-wise, Wo row-wise. Each core emits a partial
out-projection [S, E]; the host sums each pair of partials (the "all-reduce")
and adds bo.

Per-core kernel (all matmuls bf16, fp32 accumulation):
  - host supplies x[b].T so Q^T,K^T ([d,s]) and V ([s,d]) come straight off
    the projections with no on-chip transposes
  - scores computed transposed (S^T = K Q^T, [keys, queries]) with causal
    block-skipping; exp on ScalarE with fused 1/sqrt(D) scale reads PSUM
    directly and writes bf16
  - softmax denominator comes free from a ones-column appended to V in the
    attn @ V matmul; normalization is deferred to the [d, s] attention output
    (rank-1 PE broadcast of 1/sum + one vector multiply)
  - ScalarE is reserved for exp (the pacing engine): Q/K bias adds run on
    VectorE, and the V bias is folded into V itself before attention (exact:
    softmax rows sum to 1, so attn @ (V + 1 b^T) = attn @ V + 1 b^T)
  - softmax reciprocal uses the fast custom-DVE approximation (~5x faster
    than InstReciprocal, which at ~4us/call re-throttled the PE clock at
    every phase boundary)
  - ~20 warm-up matmuls on a constant tile bring the PE HAM clock to
    2.4 GHz during the initial input-DMA window
"""

import numpy as np
import ml_dtypes

B, S, E, H, D = 4, 2048, 1024, 16, 64
HPC = 8          # heads per core
DC = HPC * D     # 512 sharded feature cols per core
EC = E // 128    # 8 e-chunks
TT = S // 128    # 16 token tiles
QCH = S // 512   # 4 query chunks
NB = S // 128    # 16 key blocks

BF16 = ml_dtypes.bfloat16

_CACHE = {}


def _build():
    import concourse.tile as tile
    from concourse import bacc, mybir

    F32 = mybir.dt.float32
    BF = mybir.dt.bfloat16
    AF = mybir.ActivationFunctionType
    ALU = mybir.AluOpType

    nc = bacc.Bacc("TRN2", target_bir_lowering=False, debug=False, num_devices=8)

    xT_d = nc.dram_tensor("xT", [EC, 128, S], BF, kind="ExternalInput")
    wq_d = nc.dram_tensor("wq", [EC, 128, DC], BF, kind="ExternalInput")
    wk_d = nc.dram_tensor("wk", [EC, 128, DC], BF, kind="ExternalInput")
    wv_d = nc.dram_tensor("wv", [EC, 128, DC], BF, kind="ExternalInput")
    wo_d = nc.dram_tensor("wo", [DC // 128, 128, E], BF, kind="ExternalInput")
    bq_d = nc.dram_tensor("bq", [128, 4], F32, kind="ExternalInput")
    bk_d = nc.dram_tensor("bk", [128, 4], F32, kind="ExternalInput")
    bvb_d = nc.dram_tensor("bvb", [128, DC], F32, kind="ExternalInput")
    mask_d = nc.dram_tensor("mask", [128, 128], BF, kind="ExternalInput")
    out_d = nc.dram_tensor("out", [TT, 128, E], F32, kind="ExternalOutput")

    with tile.TileContext(nc) as tc:
        with tc.tile_pool(name="const", bufs=1) as cp, \
             tc.tile_pool(name="expp", bufs=4) as expp, \
             tc.tile_pool(name="work", bufs=2) as wp, \
             tc.tile_pool(name="ps_s", bufs=2, space="PSUM") as ps_s, \
             tc.tile_pool(name="ps_av", bufs=2, space="PSUM") as ps_av, \
             tc.tile_pool(name="ps_w", bufs=2, space="PSUM") as ps_w:

            # ---- PE warm-up: hold the HAM clock at 2.4 GHz through the
            # input-DMA window (PE would otherwise idle cold for ~20us) ----
            wu = cp.tile([64, 512], BF, tag="wu", name="wu")
            nc.vector.memset(wu[:], 0.125)
            for _ in range(20):
                pw = ps_w.tile([128, 512], F32, tag="psw", name="psw")
                nc.tensor.matmul(pw[0:64, :], wu[:, 0:64], wu[:],
                                 start=True, stop=True)

            # ---- persistent SBUF tensors + input DMAs ----
            xT = [cp.tile([128, S], BF, tag=f"xT{k}", name=f"xT{k}") for k in range(EC)]
            wq = [cp.tile([128, DC], BF, tag=f"wq{k}", name=f"wq{k}") for k in range(EC)]
            wk = [cp.tile([128, DC], BF, tag=f"wk{k}", name=f"wk{k}") for k in range(EC)]
            wv = [cp.tile([128, DC], BF, tag=f"wv{k}", name=f"wv{k}") for k in range(EC)]
            wo = [cp.tile([128, E], BF, tag=f"wo{k}", name=f"wo{k}") for k in range(DC // 128)]
            for k in range(EC):
                nc.sync.dma_start(xT[k][:], xT_d.ap()[k])
                nc.gpsimd.dma_start(wq[k][:], wq_d.ap()[k])
                nc.gpsimd.dma_start(wk[k][:], wk_d.ap()[k])
                nc.gpsimd.dma_start(wv[k][:], wv_d.ap()[k])
            for k in range(DC // 128):
                nc.sync.dma_start(wo[k][:], wo_d.ap()[k])
            bq = cp.tile([128, 4], F32, tag="bq", name="bq")
            bk = cp.tile([128, 4], F32, tag="bk", name="bk")
            bvb = cp.tile([128, DC], F32, tag="bvb", name="bvb")
            mask = cp.tile([128, 128], BF, tag="mask", name="mask")
            nc.sync.dma_start(bq[:], bq_d.ap())
            nc.sync.dma_start(bk[:], bk_d.ap())
            nc.sync.dma_start(bvb[:], bvb_d.ap())
            nc.sync.dma_start(mask[:], mask_d.ap())
            ones = cp.tile([65, 64], BF, tag="ones", name="ones")
            nc.any.memset(ones[:], 1.0)

            QT = [cp.tile([128, S], BF, tag=f"QT{t}", name=f"QT{t}") for t in range(4)]
            KT = [cp.tile([128, S], BF, tag=f"KT{t}", name=f"KT{t}") for t in range(4)]
            V = [cp.tile([128, HPC, 66], BF, tag=f"V{s}", name=f"V{s}") for s in range(TT)]
            AOT = [cp.tile([128, S], BF, tag=f"AOT{t}", name=f"AOT{t}") for t in range(4)]

            # Projection work is emitted as "filler" interleaved into the
            # attention stream: attention is ScalarE(exp)-paced with PE half
            # idle, while projections are pure dense PE - mixing them keeps
            # every engine busy. Fillers are forced ahead of their consumers.
            filler = []

            def proj_group(w_sb, b_sb, dst, t, qc):
                def emit():
                    ps = ps_w.tile([128, 512], F32, tag="psw", name="psw")
                    for k in range(EC):
                        nc.tensor.matmul(
                            ps[:],
                            w_sb[k][:, t * 128:(t + 1) * 128],
                            xT[k][:, qc * 512:(qc + 1) * 512],
                            start=(k == 0), stop=(k == EC - 1))
                    nc.vector.tensor_scalar(
                        dst[t][:, qc * 512:(qc + 1) * 512], ps[:],
                        b_sb[:, t:t + 1], None, ALU.add)
                return emit

            def v_group(s):
                def emit():
                    ps = ps_w.tile([128, 512], F32, tag="psw", name="psw")
                    for k in range(EC):
                        nc.tensor.matmul(
                            ps[:],
                            xT[k][:, s * 128:(s + 1) * 128],
                            wv[k][:],
                            start=(k == 0), stop=(k == EC - 1))
                    nc.vector.tensor_tensor(
                        V[s][:, :, 0:64],
                        ps[:].rearrange("p (h d) -> p h d", d=64),
                        bvb[:].rearrange("p (h d) -> p h d", d=64),
                        ALU.add)
                    nc.any.memset(V[s][:, :, 64:65], 1.0)
                return emit

            def d_group(s):
                def emit():
                    osb = wp.tile([128, E], F32, tag="osb", name="osb")
                    for n in range(2):
                        ps = ps_w.tile([128, 512], F32, tag="psw", name="psw")
                        for k in range(DC // 128):
                            nc.tensor.matmul(
                                ps[:],
                                AOT[k][:, s * 128:(s + 1) * 128],
                                wo[k][:, n * 512:(n + 1) * 512],
                                start=(k == 0), stop=(k == DC // 128 - 1))
                        nc.vector.tensor_copy(out=osb[:, n * 512:(n + 1) * 512],
                                              in_=ps[:])
                    nc.sync.dma_start(out_d.ap()[s], osb[:])
                return emit

            # up-front: pair-0 projections + first V tiles (attention prologue)
            for t in range(4):
                for qc in range(QCH):
                    if t == 0:
                        proj_group(wq, bq, QT, t, qc)()
                        proj_group(wk, bk, KT, t, qc)()
                    else:
                        filler.append(("qkt", t, proj_group(wq, bq, QT, t, qc)))
                        filler.append(("qkt", t, proj_group(wk, bk, KT, t, qc)))
            for s in range(TT):
                if s < 4:
                    v_group(s)()
                else:
                    filler.append(("v", s, v_group(s)))

            def emit_filler_until(pred_drop):
                """Emit (and drop) all fillers matching pred_drop."""
                keep = []
                for item in filler:
                    if pred_drop(item):
                        item[2]()
                    else:
                        keep.append(item)
                filler[:] = keep

            def emit_some_filler(n):
                for _ in range(min(n, len(filler))):
                    filler.pop(0)[2]()

            # ---- attention, head-pair interleaved, qc-outer ----
            # qc=3 (the longest phase, 32 rounds) runs first so it absorbs
            # the projection fillers; V tiles are forced in per-round just
            # before the key block that consumes them.
            for qc in (3, 0, 1, 2):
                nkb = 4 * qc + 4
                for hp in range(4):
                    emit_filler_until(lambda it: it[0] == "qkt" and it[1] <= hp)
                    hA, hB = 2 * hp, 2 * hp + 1
                    pav = {}
                    pav[hA] = ps_av.tile([128, 512], F32, tag="pav", name="pav")
                    pav[hB] = ps_av.tile([128, 512], F32, tag="pav", name="pav")

                    def emit_av(ex, kbs_offs):
                        for h in (hA, hB):
                            for i, kb, off in kbs_offs:
                                nc.tensor.matmul(
                                    pav[h][0:65, off:512],
                                    V[kb][:, h, 0:65],
                                    ex[h][:, i, off:512],
                                    start=(kb == 0), stop=(kb == nkb - 1))

                    for s0 in range(0, nkb, 2):
                        kbs = list(range(s0, min(s0 + 2, nkb)))
                        emit_filler_until(
                            lambda it: it[0] == "v" and it[1] <= kbs[-1])
                        pss = {h: ps_s.tile([128, 2, 512], F32, tag="pss",
                                            name="pss")
                               for h in (hA, hB)}
                        ex = {h: expp.tile([128, 2, 512], BF,
                                           tag=f"ex{h % 2}", name="ex")
                              for h in (hA, hB)}
                        offs = {}
                        for i, kb in enumerate(kbs):
                            dj = kb - 4 * qc
                            off = 128 * dj if dj > 0 else 0
                            offs[kb] = off
                            for h, r in ((hA, 0), (hB, 64)):
                                nc.tensor.matmul(
                                    pss[h][:, i, off:512],
                                    KT[hp][r:r + 64, kb * 128:(kb + 1) * 128],
                                    QT[hp][r:r + 64,
                                           qc * 512 + off:(qc + 1) * 512],
                                    start=True, stop=True)
                        for h in (hA, hB):
                            if kbs[-1] < 4 * qc:
                                nc.scalar.activation(
                                    ex[h][:, 0:len(kbs), :],
                                    pss[h][:, 0:len(kbs), :],
                                    AF.Exp, scale=0.125)
                            else:
                                for i, kb in enumerate(kbs):
                                    dj = kb - 4 * qc
                                    off = offs[kb]
                                    nc.scalar.activation(
                                        ex[h][:, i, off:512],
                                        pss[h][:, i, off:512],
                                        AF.Exp, scale=0.125)
                                    if dj >= 0:
                                        nc.vector.tensor_tensor(
                                            ex[h][:, i, off:off + 128],
                                            ex[h][:, i, off:off + 128],
                                            mask[:], ALU.mult)
                        emit_av(ex, [(i, kb, offs[kb])
                                     for i, kb in enumerate(kbs)])
                        emit_some_filler(1)
                    # normalization per head (V-bias already folded into V)
                    for h, r in ((hA, 0), (hB, 64)):
                        rcp = wp.tile([65, 512], BF, tag="rcp", name="rcp")
                        with nc.allow_low_precision("softmax denom bcast"):
                            nc.vector.reciprocal(out=rcp[64:65, :],
                                                 in_=pav[h][64:65, :])
                        psb = ps_w.tile([128, 512], F32, tag="psw", name="psw")
                        nc.tensor.matmul(psb[0:64, :], ones[64:65, :],
                                         rcp[64:65, :], start=True, stop=True)
                        binv = wp.tile([64, 512], F32, tag="binv", name="binv")
                        nc.vector.tensor_copy(out=binv[:], in_=psb[0:64, :])
                        dst = AOT[hp][r:r + 64, qc * 512:(qc + 1) * 512]
                        nc.vector.tensor_tensor(dst, pav[h][0:64, :],
                                                binv[:], ALU.mult)
                # out-projection for this qc becomes filler for qc+1
                for s in range(qc * 4, qc * 4 + 4):
                    filler.append(("d", s, d_group(s)))
            emit_filler_until(lambda it: True)

    nc.compile()
    return nc


def _get_nc():
    if "nc" not in _CACHE:
        _CACHE["nc"] = _build()
    return _CACHE["nc"]


def _shard_inputs(x, Wq, bq, Wk, bk, Wv, bv, Wo):
    """Build the 8 per-core input maps (host-side shard/cast/transpose)."""
    x = np.asarray(x, np.float32)
    mask = np.triu(np.ones((128, 128), np.float32)).astype(BF16)  # [k, q] q>=k
    in_maps = []
    for c in range(8):
        b, hg = divmod(c, 2)
        dc = slice(hg * DC, (hg + 1) * DC)
        xT = np.ascontiguousarray(x[b].T).astype(BF16).reshape(EC, 128, S)
        wq_c = np.ascontiguousarray(Wq[:, dc]).astype(BF16).reshape(EC, 128, DC)
        wk_c = np.ascontiguousarray(Wk[:, dc]).astype(BF16).reshape(EC, 128, DC)
        wv_c = np.ascontiguousarray(Wv[:, dc]).astype(BF16).reshape(EC, 128, DC)
        wo_c = np.ascontiguousarray(Wo[dc, :]).astype(BF16).reshape(DC // 128, 128, E)
        bq_c = np.ascontiguousarray(np.asarray(bq[dc], np.float32).reshape(4, 128).T)
        bk_c = np.ascontiguousarray(np.asarray(bk[dc], np.float32).reshape(4, 128).T)
        bvb_c = np.ascontiguousarray(
            np.tile(np.asarray(bv[dc], np.float32).reshape(1, DC), (128, 1)))
        in_maps.append({
            "xT": xT, "wq": wq_c, "wk": wk_c, "wv": wv_c, "wo": wo_c,
            "bq": bq_c, "bk": bk_c, "bvb": bvb_c, "mask": mask,
        })
    return in_maps


def kernel(x, Wq, bq, Wk, bk, Wv, bv, Wo, bo):
    from concourse.bass_utils import run_bass_kernel_spmd

    nc = _get_nc()
    in_maps = _shard_inputs(x, Wq, bq, Wk, bk, Wv, bv, Wo)
    res = run_bass_kernel_spmd(nc, in_maps, core_ids=list(range(8)))
    bo = np.asarray(bo, np.float32)
    out = np.empty((B, S, E), np.float32)
    for b in range(B):
        p0 = res.results[2 * b]["out"].reshape(S, E)
        p1 = res.results[2 * b + 1]["out"].reshape(S, E)
        out[b] = p0 + p1 + bo
    return out


# revision 5
# speedup vs baseline: 1.4784x; 1.2466x over previous
"""Multi-head self-attention (B=4, S=2048, E=1024, H=16, causal) on 8 NeuronCores.

Sharding (Megatron-style, per hint): data-parallel over B (4) x tensor-parallel
over heads (2 groups of 8 heads). Core c handles batch c//2 with head-group
c%2: Wq/Wk/Wv sharded column-wise, Wo row-wise. Each core emits a partial
out-projection [S, E]; the host sums each pair of partials (the "all-reduce")
and adds bo.

Per-core kernel (all matmuls bf16, fp32 accumulation):
  - host supplies x[b].T so Q^T,K^T ([d,s]) and V ([s,d]) come straight off
    the projections with no on-chip transposes
  - scores computed transposed (S^T = K Q^T, [keys, queries]) with causal
    block-skipping; exp on ScalarE with fused 1/sqrt(D) scale reads PSUM
    directly and writes bf16
  - softmax denominator comes free from a ones-column appended to V in the
    attn @ V matmul; normalization is deferred to the [d, s] attention output
    (rank-1 PE broadcast of 1/sum + one vector multiply)
  - ScalarE is reserved for exp (the pacing engine): Q/K bias adds run on
    VectorE, and the V bias is folded into V itself before attention (exact:
    softmax rows sum to 1, so attn @ (V + 1 b^T) = attn @ V + 1 b^T)
  - softmax reciprocal uses the fast custom-DVE approximation (~5x faster
    than InstReciprocal, which at ~4us/call re-throttled the PE clock at
    every phase boundary)
  - ~20 warm-up matmuls on a constant tile bring the PE HAM clock to
    2.4 GHz during the initial input-DMA window
"""

import numpy as np
import ml_dtypes

B, S, E, H, D = 4, 2048, 1024, 16, 64
HPC = 8          # heads per core
DC = HPC * D     # 512 sharded feature cols per core
EC = E // 128    # 8 e-chunks
TT = S // 128    # 16 token tiles
QCH = S // 512   # 4 query chunks
NB = S // 128    # 16 key blocks

BF16 = ml_dtypes.bfloat16

_CACHE = {}


def _build():
    import concourse.tile as tile
    from concourse import bacc, mybir

    F32 = mybir.dt.float32
    BF = mybir.dt.bfloat16
    AF = mybir.ActivationFunctionType
    ALU = mybir.AluOpType

    nc = bacc.Bacc("TRN2", target_bir_lowering=False, debug=False, num_devices=8)

    xT_d = nc.dram_tensor("xT", [EC, 128, S], BF, kind="ExternalInput")
    wq_d = nc.dram_tensor("wq", [EC, 128, DC], BF, kind="ExternalInput")
    wk_d = nc.dram_tensor("wk", [EC, 128, DC], BF, kind="ExternalInput")
    wv_d = nc.dram_tensor("wv", [EC, 128, DC], BF, kind="ExternalInput")
    wo_d = nc.dram_tensor("wo", [DC // 128, 128, E], BF, kind="ExternalInput")
    bq_d = nc.dram_tensor("bq", [128, 4], F32, kind="ExternalInput")
    bk_d = nc.dram_tensor("bk", [128, 4], F32, kind="ExternalInput")
    bvb_d = nc.dram_tensor("bvb", [128, DC], F32, kind="ExternalInput")
    mask_d = nc.dram_tensor("mask", [128, 128], BF, kind="ExternalInput")
    out_d = nc.dram_tensor("out", [TT, 128, E], F32, kind="ExternalOutput")

    with tile.TileContext(nc) as tc:
        with tc.tile_pool(name="const", bufs=1) as cp, \
             tc.tile_pool(name="expp", bufs=4) as expp, \
             tc.tile_pool(name="work", bufs=2) as wp, \
             tc.tile_pool(name="ps_s", bufs=2, space="PSUM") as ps_s, \
             tc.tile_pool(name="ps_av", bufs=2, space="PSUM") as ps_av, \
             tc.tile_pool(name="ps_w", bufs=2, space="PSUM") as ps_w:

            # ---- PE warm-up: hold the HAM clock at 2.4 GHz through the
            # input-DMA window (PE would otherwise idle cold for ~20us) ----
            wu = cp.tile([64, 512], BF, tag="wu", name="wu")
            nc.vector.memset(wu[:], 0.125)
            for _ in range(20):
                pw = ps_w.tile([128, 512], F32, tag="psw", name="psw")
                nc.tensor.matmul(pw[0:64, :], wu[:, 0:64], wu[:],
                                 start=True, stop=True)

            # ---- persistent SBUF tensors + input DMAs ----
            xT = [cp.tile([128, S], BF, tag=f"xT{k}", name=f"xT{k}") for k in range(EC)]
            wq = [cp.tile([128, DC], BF, tag=f"wq{k}", name=f"wq{k}") for k in range(EC)]
            wk = [cp.tile([128, DC], BF, tag=f"wk{k}", name=f"wk{k}") for k in range(EC)]
            wv = [cp.tile([128, DC], BF, tag=f"wv{k}", name=f"wv{k}") for k in range(EC)]
            wo = [cp.tile([128, E], BF, tag=f"wo{k}", name=f"wo{k}") for k in range(DC // 128)]
            for k in range(EC):
                nc.sync.dma_start(xT[k][:], xT_d.ap()[k])
                nc.gpsimd.dma_start(wq[k][:], wq_d.ap()[k])
                nc.gpsimd.dma_start(wk[k][:], wk_d.ap()[k])
                nc.gpsimd.dma_start(wv[k][:], wv_d.ap()[k])
            for k in range(DC // 128):
                nc.sync.dma_start(wo[k][:], wo_d.ap()[k])
            bq = cp.tile([128, 4], F32, tag="bq", name="bq")
            bk = cp.tile([128, 4], F32, tag="bk", name="bk")
            bvb = cp.tile([128, DC], F32, tag="bvb", name="bvb")
            mask = cp.tile([128, 128], BF, tag="mask", name="mask")
            nc.sync.dma_start(bq[:], bq_d.ap())
            nc.sync.dma_start(bk[:], bk_d.ap())
            nc.sync.dma_start(bvb[:], bvb_d.ap())
            nc.sync.dma_start(mask[:], mask_d.ap())
            ones = cp.tile([65, 64], BF, tag="ones", name="ones")
            nc.any.memset(ones[:], 1.0)

            QT = [cp.tile([128, S], BF, tag=f"QT{t}", name=f"QT{t}") for t in range(4)]
            KT = [cp.tile([128, S], BF, tag=f"KT{t}", name=f"KT{t}") for t in range(4)]
            V = [cp.tile([128, HPC, 66], BF, tag=f"V{s}", name=f"V{s}") for s in range(TT)]
            AOT = [cp.tile([128, S], BF, tag=f"AOT{t}", name=f"AOT{t}") for t in range(4)]

            # Projection work is emitted as "filler" interleaved into the
            # attention stream: attention is ScalarE(exp)-paced with PE half
            # idle, while projections are pure dense PE - mixing them keeps
            # every engine busy. Fillers are forced ahead of their consumers.
            filler = []

            def proj_group(w_sb, b_sb, dst, t, qc):
                def emit():
                    ps = ps_w.tile([128, 512], F32, tag="psw", name="psw")
                    for k in range(EC):
                        nc.tensor.matmul(
                            ps[:],
                            w_sb[k][:, t * 128:(t + 1) * 128],
                            xT[k][:, qc * 512:(qc + 1) * 512],
                            start=(k == 0), stop=(k == EC - 1))
                    nc.vector.tensor_scalar(
                        dst[t][:, qc * 512:(qc + 1) * 512], ps[:],
                        b_sb[:, t:t + 1], None, ALU.add)
                return emit

            def v_group(s):
                def emit():
                    ps = ps_w.tile([128, 512], F32, tag="psw", name="psw")
                    for k in range(EC):
                        nc.tensor.matmul(
                            ps[:],
                            xT[k][:, s * 128:(s + 1) * 128],
                            wv[k][:],
                            start=(k == 0), stop=(k == EC - 1))
                    nc.vector.tensor_tensor(
                        V[s][:, :, 0:64],
                        ps[:].rearrange("p (h d) -> p h d", d=64),
                        bvb[:].rearrange("p (h d) -> p h d", d=64),
                        ALU.add)
                    nc.any.memset(V[s][:, :, 64:65], 1.0)
                return emit

            def d_group(s):
                def emit():
                    osb = wp.tile([128, E], F32, tag="osb", name="osb")
                    for n in range(2):
                        ps = ps_w.tile([128, 512], F32, tag="psw", name="psw")
                        for k in range(DC // 128):
                            nc.tensor.matmul(
                                ps[:],
                                AOT[k][:, s * 128:(s + 1) * 128],
                                wo[k][:, n * 512:(n + 1) * 512],
                                start=(k == 0), stop=(k == DC // 128 - 1))
                        nc.vector.tensor_copy(out=osb[:, n * 512:(n + 1) * 512],
                                              in_=ps[:])
                    nc.sync.dma_start(out_d.ap()[s], osb[:])
                return emit

            # up-front: pair-0 projections + first V tiles (attention prologue)
            for t in range(4):
                for qc in range(QCH):
                    if t == 0:
                        proj_group(wq, bq, QT, t, qc)()
                        proj_group(wk, bk, KT, t, qc)()
                    else:
                        filler.append(("qkt", t, proj_group(wq, bq, QT, t, qc)))
                        filler.append(("qkt", t, proj_group(wk, bk, KT, t, qc)))
            for s in range(TT):
                if s < 4:
                    v_group(s)()
                else:
                    filler.append(("v", s, v_group(s)))

            def emit_filler_until(pred_drop):
                """Emit (and drop) all fillers matching pred_drop."""
                keep = []
                for item in filler:
                    if pred_drop(item):
                        item[2]()
                    else:
                        keep.append(item)
                filler[:] = keep

            def emit_some_filler(n):
                for _ in range(min(n, len(filler))):
                    filler.pop(0)[2]()

            # ---- attention, head-pair interleaved, qc-outer ----
            # qc=3 (the longest phase, 32 rounds) runs first so it absorbs
            # the projection fillers; V tiles are forced in per-round just
            # before the key block that consumes them.
            for qc in (3, 0, 1, 2):
                nkb = 4 * qc + 4
                for hp in range(4):
                    emit_filler_until(lambda it: it[0] == "qkt" and it[1] <= hp)
                    hA, hB = 2 * hp, 2 * hp + 1
                    pav = {}
                    pav[hA] = ps_av.tile([128, 512], F32, tag="pav", name="pav")
                    pav[hB] = ps_av.tile([128, 512], F32, tag="pav", name="pav")

                    def emit_av(ex, kbs_offs):
                        for h in (hA, hB):
                            for i, kb, off in kbs_offs:
                                nc.tensor.matmul(
                                    pav[h][0:65, off:512],
                                    V[kb][:, h, 0:65],
                                    ex[h][:, i, off:512],
                                    start=(kb == 0), stop=(kb == nkb - 1))

                    # AV runs one round behind scores/exp: by the time the PE
                    # reaches AV(r-1), its exp(r-1) finished during round r's
                    # scores+filler, so the in-order PE queue never stalls on
                    # ScalarE (stalls re-throttle the HAM clock to 1.2 GHz).
                    pend = None
                    for s0 in range(0, nkb, 2):
                        kbs = list(range(s0, min(s0 + 2, nkb)))
                        emit_filler_until(
                            lambda it: it[0] == "v" and it[1] <= kbs[-1])
                        pss = {h: ps_s.tile([128, 2, 512], F32, tag="pss",
                                            name="pss")
                               for h in (hA, hB)}
                        ex = {h: expp.tile([128, 2, 512], BF,
                                           tag=f"ex{h % 2}", name="ex")
                              for h in (hA, hB)}
                        offs = {}
                        for i, kb in enumerate(kbs):
                            dj = kb - 4 * qc
                            off = 128 * dj if dj > 0 else 0
                            offs[kb] = off
                            for h, r in ((hA, 0), (hB, 64)):
                                nc.tensor.matmul(
                                    pss[h][:, i, off:512],
                                    KT[hp][r:r + 64, kb * 128:(kb + 1) * 128],
                                    QT[hp][r:r + 64,
                                           qc * 512 + off:(qc + 1) * 512],
                                    start=True, stop=True)
                        for h in (hA, hB):
                            if kbs[-1] < 4 * qc:
                                nc.scalar.activation(
                                    ex[h][:, 0:len(kbs), :],
                                    pss[h][:, 0:len(kbs), :],
                                    AF.Exp, scale=0.125)
                            else:
                                for i, kb in enumerate(kbs):
                                    dj = kb - 4 * qc
                                    off = offs[kb]
                                    nc.scalar.activation(
                                        ex[h][:, i, off:512],
                                        pss[h][:, i, off:512],
                                        AF.Exp, scale=0.125)
                                    if dj >= 0:
                                        nc.vector.tensor_tensor(
                                            ex[h][:, i, off:off + 128],
                                            ex[h][:, i, off:off + 128],
                                            mask[:], ALU.mult)
                        emit_some_filler(1)
                        if pend is not None:
                            emit_av(*pend)
                        pend = (ex, [(i, kb, offs[kb])
                                     for i, kb in enumerate(kbs)])
                    emit_av(*pend)
                    # normalization per head (V-bias already folded into V).
                    # reciprocal_approx_fast only works at base partition 0 on
                    # HW, so broadcast the denominator first (1-channel aligned
                    # move + rank-1 PE broadcast), then invert the broadcast.
                    for h, r in ((hA, 0), (hB, 64)):
                        den = wp.tile([1, 512], BF, tag="den", name="den")
                        nc.vector.tensor_copy(out=den[:],
                                              in_=pav[h][64:65, :])
                        psb = ps_w.tile([128, 512], F32, tag="psw", name="psw")
                        nc.tensor.matmul(psb[0:64, :], ones[0:1, :],
                                         den[:], start=True, stop=True)
                        denb = wp.tile([64, 512], F32, tag="denb", name="denb")
                        nc.vector.tensor_copy(out=denb[:], in_=psb[0:64, :])
                        rcpb = wp.tile([64, 512], F32, tag="rcpb", name="rcpb")
                        nc.vector.reciprocal_approx_fast(out=rcpb[:],
                                                         in_=denb[:])
                        dst = AOT[hp][r:r + 64, qc * 512:(qc + 1) * 512]
                        nc.vector.tensor_tensor(dst, pav[h][0:64, :],
                                                rcpb[:], ALU.mult)
                # out-projection for this qc becomes filler for qc+1
                for s in range(qc * 4, qc * 4 + 4):
                    filler.append(("d", s, d_group(s)))
            emit_filler_until(lambda it: True)

    nc.compile()
    return nc


def _get_nc():
    if "nc" not in _CACHE:
        _CACHE["nc"] = _build()
    return _CACHE["nc"]


def _shard_inputs(x, Wq, bq, Wk, bk, Wv, bv, Wo):
    """Build the 8 per-core input maps (host-side shard/cast/transpose)."""
    x = np.asarray(x, np.float32)
    mask = np.triu(np.ones((128, 128), np.float32)).astype(BF16)  # [k, q] q>=k
    in_maps = []
    for c in range(8):
        b, hg = divmod(c, 2)
        dc = slice(hg * DC, (hg + 1) * DC)
        xT = np.ascontiguousarray(x[b].T).astype(BF16).reshape(EC, 128, S)
        wq_c = np.ascontiguousarray(Wq[:, dc]).astype(BF16).reshape(EC, 128, DC)
        wk_c = np.ascontiguousarray(Wk[:, dc]).astype(BF16).reshape(EC, 128, DC)
        wv_c = np.ascontiguousarray(Wv[:, dc]).astype(BF16).reshape(EC, 128, DC)
        wo_c = np.ascontiguousarray(Wo[dc, :]).astype(BF16).reshape(DC // 128, 128, E)
        bq_c = np.ascontiguousarray(np.asarray(bq[dc], np.float32).reshape(4, 128).T)
        bk_c = np.ascontiguousarray(np.asarray(bk[dc], np.float32).reshape(4, 128).T)
        bvb_c = np.ascontiguousarray(
            np.tile(np.asarray(bv[dc], np.float32).reshape(1, DC), (128, 1)))
        in_maps.append({
            "xT": xT, "wq": wq_c, "wk": wk_c, "wv": wv_c, "wo": wo_c,
            "bq": bq_c, "bk": bk_c, "bvb": bvb_c, "mask": mask,
        })
    return in_maps


def kernel(x, Wq, bq, Wk, bk, Wv, bv, Wo, bo):
    from concourse.bass_utils import run_bass_kernel_spmd

    nc = _get_nc()
    in_maps = _shard_inputs(x, Wq, bq, Wk, bk, Wv, bv, Wo)
    res = run_bass_kernel_spmd(nc, in_maps, core_ids=list(range(8)))
    bo = np.asarray(bo, np.float32)
    out = np.empty((B, S, E), np.float32)
    for b in range(B):
        p0 = res.results[2 * b]["out"].reshape(S, E)
        p1 = res.results[2 * b + 1]["out"].reshape(S, E)
        out[b] = p0 + p1 + bo
    return out
